# revision 22
# baseline (speedup 1.0000x reference)
"""Trainium2 Bass kernel for nn_KANModel (KAN recommender).

Math: with the shared uniform grid (G=5, k=3), cubic B-spline bases on the
extended uniform knots are shifted cardinal splines, so each KAN layer is a
sum of relu(u-n)^3 blocks folded into per-block matmul weights (see v1).
Two structural collapses on top of that:

  * layer 0: the gathered embeddings x span [x_min, x_max], so
    u0 = (x-t0)/h stays in ~[4.1, 6.8]. Blocks with n <= floor(u0_min)
    never clamp -> their relu cubes sum to a single cubic polynomial in
    v = u0 - vc, evaluated as three matmuls (v, v^2, v^3) with host-folded
    coefficients; only the top blocks keep their relu. silu(x) rides the
    same accumulation via sigmoid(x)*(v*h + t0v) split into two matmuls.

  * layer 0 outputs are produced feature-major and DUPLICATED across both
    partition halves (lhsT = [W | W]), pre-scaled by 1/h1. Layer 1's 12
    relu^3 blocks then pack two-per-partition-group: 6 elementwise ops with
    per-partition constant vectors + 6 f32 matmuls contracting (k,o) pairs
    over partitions into y[b].

Layer 0 runs in fp16 (verified ~7e-4 final rel err); layer 1 keeps f32
(r^3 reaches ~3e3 with heavy cancellation - fp16 fails the 2e-2 gate).
Embedding rows are fetched with ONE 256-descriptor indirect DMA from a
host-concatenated [NU+NI, 64] table (one SWDGE generation pass instead of
two serialized ones). Data-parallel over batch: 1024 rows -> 8 cores x 128.
"""

import numpy as np

B_FULL = 1024
NCORES = 8
BS = B_FULL // NCORES          # batch shard per core
D = 64                         # embedding dim
IN0, OUT0 = 2 * D, 64          # KAN layer 0
IN1 = 64                       # KAN layer 1 (out_dim 1)
G, KORD = 5, 3
NC_BASIS = G + KORD            # 8 spline bases per edge
NZ = G + 2 * KORD + 1          # 12 possible relu-cube shifts
NU, NI = 100000, 50000

_BUILD_CACHE = {}
TRACE = False
LAST_RESULTS = None

_A5 = np.array([1.0, -4.0, 6.0, -4.0, 1.0], dtype=np.float64) / 6.0


def _m3(s):
    """Cardinal cubic B-spline, exact (clamped) evaluation, float64."""
    s = np.minimum(s, 4.0)
    out = np.zeros_like(s)
    for m in range(4):
        r = np.maximum(s - m, 0.0)
        out += _A5[m] * r * r * r
    return out


def _fold_host_weights(grid0, coef0, sb0, ssp0, bias0, grid1, coef1, sb1, ssp1,
                       bias1, x_min, x_max):
    """O(params) host-side prep: folded weights with the cubic collapse for
    never-clamping blocks, duplicated feature-major layouts, and the packed
    layer-1 constant vectors."""
    h0 = float(grid0[0, -1] - grid0[0, 0]) / G
    t0_0 = float(grid0[0, 0]) - KORD * h0
    h1 = float(grid1[0, -1] - grid1[0, 0]) / G
    t0_1 = float(grid1[0, 0]) - KORD * h1
    inv_h1 = 1.0 / h1

    u0_min = (x_min - t0_0) / h0
    u0_max = (x_max - t0_0) / h0
    ncut = int(np.floor(u0_min + 1e-9))          # blocks 0..ncut never clamp
    vc = 0.5 * (u0_min + u0_max)                  # centering for fp16 powers
    t0v = t0_0 + vc * h0                          # v = (x - t0v)/h0
    nrem = [n for n in range(max(ncut + 1, 0), NZ) if n < u0_max + 1e-6]

    c0e = (ssp0[:, None].astype(np.float64) * coef0.astype(np.float64)).reshape(
        OUT0, IN0, NC_BASIS
    )  # (o, f, c)
    wz0 = np.zeros((NZ, IN0, OUT0), dtype=np.float64)
    for n in range(NZ):
        for m in range(5):
            c = n - m
            if 0 <= c < NC_BASIS:
                wz0[n] += _A5[m] * c0e[:, :, c].T

    # cubic collapse: sum_{n<=ncut} w_n (v + (vc-n))^3 -> powers of v
    cpoly = np.zeros((4, IN0, OUT0), dtype=np.float64)
    for n in range(ncut + 1):
        a = vc - n
        cpoly[0] += wz0[n] * a ** 3
        cpoly[1] += wz0[n] * (3 * a * a)
        cpoly[2] += wz0[n] * (3 * a)
        cpoly[3] += wz0[n]

    sb0e = sb0.reshape(OUT0, IN0).astype(np.float64)  # (o, f)

    def dup(w_fo):  # [f, OUT0] -> [f, 128] duplicated across halves
        return np.concatenate([w_fo, w_fo], axis=1)

    blocks = [dup(cpoly[1] * inv_h1), dup(cpoly[2] * inv_h1),
              dup(cpoly[3] * inv_h1)]
    for n in nrem:
        blocks.append(dup(wz0[n] * inv_h1))
    blocks.append(dup(sb0e.T * (h0 * inv_h1)))    # rhs = sg*v
    blocks.append(dup(sb0e.T * (t0v * inv_h1)))   # rhs = sg
    blob = np.ascontiguousarray(
        np.concatenate(blocks, axis=1).astype(np.float16))

    c0sum = cpoly[0].sum(axis=0)                  # constant term per o
    biasrow = np.ascontiguousarray(
        (np.tile(bias0.astype(np.float64) + c0sum, 2) * inv_h1)
        .reshape(1, 2 * OUT0).astype(np.float32))

    # ---- rigorous layer-0 output interval (grid + Lipschitz pad) ----
    NGRID = 2049
    xg = np.linspace(x_min, x_max, NGRID)
    dx = (x_max - x_min) / (NGRID - 1) if x_max > x_min else 0.0
    ug = (xg - t0_0) / h0
    basis = np.stack([_m3(ug - c) for c in range(NC_BASIS)], axis=1)
    silug = xg / (1.0 + np.exp(-xg))
    phi = sb0e[:, :, None] * silug[None, None, :] + np.einsum(
        "ofc,gc->ofg", c0e, basis
    )
    lip = np.abs(sb0e) * 1.1 + np.abs(c0e).sum(axis=2) * (0.75 / h0)
    pad = lip * dx
    h_max = bias0.astype(np.float64) + (phi.max(axis=2) + pad).sum(axis=1)
    u1_max = (float(h_max.max()) - t0_1) / h1
    nlist1 = [n for n in range(NZ) if n < u1_max + 1e-3]
    L1 = len(nlist1)
    JH = (L1 + 1) // 2                            # packed col-blocks

    c1e = ssp1[:, None].astype(np.float64) * coef1.astype(np.float64)
    wz1 = np.zeros((NZ, IN1), dtype=np.float64)
    for n in range(NZ):
        for m in range(5):
            c = n - m
            if 0 <= c < NC_BASIS:
                wz1[n] += _A5[m] * c1e[:, c]

    # Layer-1 packing pairs blocks (j, j+JH): the per-partition constant is
    # then shift(p) = t0_1/h1 + JH*(p>=64), j-independent, and folds into
    # the layer-0 bias row. Every r-block uses a PLAIN float scalar (per-
    # partition scalar APs serialize across engines in the tile scheduler).
    w1pack = np.zeros((2 * OUT0, JH), dtype=np.float64)
    for j in range(JH):
        for half in range(2):
            k = j + JH * half
            sl = slice(half * OUT0, (half + 1) * OUT0)
            if k < L1:
                w1pack[sl, j] = wz1[nlist1[k]]
    sb1h1 = np.zeros((2 * OUT0, 2), dtype=np.float64)
    sb1h1[:OUT0, 0] = sb1.astype(np.float64) * h1   # vs h2s*sg1
    sb1h1[:OUT0, 1] = sb1.astype(np.float64) * t0_1  # vs sg1 alone
    # consts layout: [w1pack JH | sb1h1 | sb1t01]
    consts = np.ascontiguousarray(
        np.concatenate([w1pack, sb1h1], axis=1).astype(np.float32))

    # fold the layer-1 shift into the bias row: h2s = u1 - JH*(p>=64)
    shift = t0_1 * inv_h1 + JH * (np.arange(2 * OUT0) >= OUT0)
    biasrow = np.ascontiguousarray(
        (biasrow.astype(np.float64) - shift).reshape(1, 2 * OUT0)
        .astype(np.float32))

    key = (
        t0v, 1.0 / h0, vc, float(h1), inv_h1, float(bias1[0]), float(t0_1),
        tuple(nrem), JH,
    )
    return key, dict(blob=blob, biasrow=biasrow, consts=consts)


def _build_program(key, ncols_blob):
    import concourse.bass as bass
    import concourse.bacc as bacc
    import concourse.mybir as mybir
    from concourse.tile import TileContext
    from concourse.masks import make_identity

    t0v, inv_h0, vc, h1, inv_h1, bias1, t0_1, nrem, JH = key
    NR = len(nrem)
    f32 = mybir.dt.float32
    f16 = mybir.dt.float16
    i32 = mybir.dt.int32
    A = mybir.AluOpType
    AF = mybir.ActivationFunctionType

    nc = bacc.Bacc("TRN2")

    # register float activation biases as const APs (mirrors Bass.__init__).
    # Only ACT uses these: const-AP biases on ACT don't hit the cross-engine
    # scalar-pointer serialization that explicit AP scalars do.
    cvals = {float(t0v), float(t0_1), float(bias1)}
    cvals.update(float(-(n - vc)) for n in nrem)      # layer-0 q' biases
    cvals.update(float(-j) for j in range(JH // 2, JH))  # layer-1 ACT relus
    for ci, val in enumerate(sorted(cvals)):
        if (f32, val) not in nc.const_aps.aps:
            t = nc.alloc_sbuf_tensor(f"kconst-{ci}", [128, 1], f32)
            nc.gpsimd.memset(t.ap(), val)
            nc.const_aps.aps[(f32, val)] = t.ap()

    d_idx = nc.dram_tensor("idx", [BS, 2], i32, kind="ExternalInput")
    d_emb = nc.dram_tensor("embc", [NU + NI, D], f16, kind="ExternalInput")
    d_blob = nc.dram_tensor("blob", [IN0, ncols_blob], f16, kind="ExternalInput")
    d_bias = nc.dram_tensor("biasrow", [1, 2 * OUT0], f32, kind="ExternalInput")
    d_cst = nc.dram_tensor("consts", [2 * OUT0, JH + 2], f32,
                           kind="ExternalInput")
    d_out = nc.dram_tensor("out", [BS, 1], f32, kind="ExternalOutput")

    with TileContext(nc) as tc:
        with (
            tc.tile_pool(name="sb", bufs=1) as P,
            tc.tile_pool(name="ps", bufs=1, space="PSUM") as PS,
        ):
            # idx first on SP: its completion gates the gather desc-gens
            idx = P.tile([BS, 2], i32, tag="idx")
            nc.sync.dma_start(out=idx[:], in_=d_idx[:])
            blob = P.tile([IN0, ncols_blob], f16, tag="blob")
            nc.sync.dma_start(out=blob[:], in_=d_blob[:])
            brow = P.tile([1, 2 * OUT0], f32, tag="brow")
            nc.sync.dma_start(out=brow[:1, :], in_=d_bias[:])
            cst = P.tile([2 * OUT0, JH + 2], f32, tag="cst")
            nc.sync.dma_start(out=cst[:], in_=d_cst[:])

            ident = P.tile([128, 128], f16, tag="ident")
            make_identity(nc, ident[:])
            ones = P.tile([1, 128], f32, tag="ones")
            nc.gpsimd.memset(ones[:1, :], 1.0)
            # dummy ACT op: charges the 1283ns activation-table load at t~1us
            # instead of on the critical path at first real use (Sigmoid set
            # also contains Square and Relu -> single load for the kernel)
            warm = P.tile([1, 1], f32, tag="warm")
            nc.scalar.activation(warm[:1, :1], ones[:1, 0:1], AF.Sigmoid)

            # two row gathers from the fp16 combined table (multi-chunk
            # offset APs are not supported by the SWDGE ucode)
            xbm = P.tile([BS, 2, D], f16, tag="xbm")
            nc.gpsimd.indirect_dma_start(
                out=xbm[:, 0, :], out_offset=None, in_=d_emb[:],
                in_offset=bass.IndirectOffsetOnAxis(ap=idx[:, 0:1], axis=0),
            )
            nc.gpsimd.indirect_dma_start(
                out=xbm[:, 1, :], out_offset=None, in_=d_emb[:],
                in_offset=bass.IndirectOffsetOnAxis(ap=idx[:, 1:2], axis=0),
            )

            # h2 accumulation [o-dup, b]; bias matmul first (operands land
            # early; PE p-state warms before the gather-dependent chain)
            h2 = PS.tile([128, BS], f32, tag="h2")
            nc.tensor.matmul(out=h2[:], lhsT=brow[:1, :], rhs=ones[:1, :],
                             start=True, stop=False)

            # per-half transposes to feature-major x^T (f16, 1 cyc/row);
            # the user half transposes while the item gather is in flight
            xT = PS.tile([128, BS], f16, tag="xT")
            nc.tensor.matmul(out=xT[0:D, :], lhsT=xbm[:, 0, :], rhs=ident[:],
                             is_transpose=True, start=True, stop=True,
                             skip_group_check=True)
            nc.tensor.matmul(out=xT[D:128, :], lhsT=xbm[:, 1, :], rhs=ident[:],
                             is_transpose=True, start=True, stop=True,
                             skip_group_check=True)

            # fp16 operands for the layer-0 matmuls; v is the ONLY xT reader.
            # DVE: v, v2, v3, r-blocks, sv, z-blocks; ACT: sg + q'-blocks
            # (q' = (v - c)^2 straight from v via const-AP bias: no r->q dep)
            v = P.tile([128, BS], f16, tag="v")
            nc.vector.tensor_scalar(v[:], xT[:], t0v, inv_h0,
                                    A.subtract, A.mult)
            sg = P.tile([128, BS], f16, tag="sg")
            nc.scalar.activation(sg[:], v[:], AF.Sigmoid, bias=t0v,
                                 scale=1.0 / inv_h0)
            v2 = P.tile([128, BS], f16, tag="v2")
            nc.vector.tensor_tensor(out=v2[:], in0=v[:], in1=v[:], op=A.mult)
            v3 = P.tile([128, BS], f16, tag="v3")
            nc.vector.tensor_tensor(out=v3[:], in0=v2[:], in1=v[:], op=A.mult)

            r56 = P.tile([128, NR * BS], f16, tag="r56")
            q56 = P.tile([128, NR * BS], f16, tag="q56")
            z56 = P.tile([128, NR * BS], f16, tag="z56")
            for i, n in enumerate(nrem):
                sl = slice(i * BS, (i + 1) * BS)
                nc.vector.tensor_scalar(r56[:, sl], v[:],
                                        float(n - vc), 0.0, A.subtract, A.max)
                nc.scalar.activation(q56[:, sl], v[:], AF.Square,
                                     bias=float(-(n - vc)))
            sv = P.tile([128, BS], f16, tag="sv")
            nc.vector.tensor_tensor(out=sv[:], in0=sg[:], in1=v[:], op=A.mult)
            for i in range(NR):
                sl = slice(i * BS, (i + 1) * BS)
                nc.vector.tensor_tensor(out=z56[:, sl], in0=q56[:, sl],
                                        in1=r56[:, sl], op=A.mult)

            # z-blocks come last in PE order: their operands are ready last
            mm_list = []
            for bi, t in enumerate([v, v2, v3]):
                mm_list.append((blob[:, bi * 128:(bi + 1) * 128], t[:]))
            mm_list.append((blob[:, (3 + NR) * 128:(4 + NR) * 128], sv[:]))
            mm_list.append((blob[:, (4 + NR) * 128:(5 + NR) * 128], sg[:]))
            for i in range(NR):
                mm_list.append((blob[:, (3 + i) * 128:(4 + i) * 128],
                                z56[:, i * BS:(i + 1) * BS]))
            for i, (w_ap, rhs_ap) in enumerate(mm_list):
                nc.tensor.matmul(out=h2[:], lhsT=w_ap, rhs=rhs_ap,
                                 start=False, stop=(i == len(mm_list) - 1))

            # ---- layer 1, packed feature-major ----
            # h2 holds u1 - JH*(p>=64). r-blocks read PSUM directly: DVE with
            # plain scalars, ACT as Relu with const-AP bias (no SBUF copy;
            # gpsimd can't read PSUM so Pool instead computes one q-chunk).
            # silu path: h = h2[:64]*h1 + t0_1, so
            # silu(h)*sb1 = (h2*sg1)*(sb1*h1) + sg1*(sb1*t0_1)
            sg1 = P.tile([OUT0, BS], f32, tag="sg1")
            nc.scalar.activation(sg1[:], h2[:OUT0, :], AF.Sigmoid,
                                 bias=float(t0_1), scale=h1)
            sv1 = P.tile([OUT0, BS], f32, tag="sv1")
            nc.vector.tensor_tensor(out=sv1[:], in0=sg1[:], in1=h2[:OUT0, :],
                                    op=A.mult)
            yps = PS.tile([BS, 1], f32, tag="yps")
            nc.tensor.matmul(out=yps[:], lhsT=sv1[:],
                             rhs=cst[:OUT0, JH:JH + 1],
                             start=True, stop=False)
            nc.tensor.matmul(out=yps[:], lhsT=sg1[:],
                             rhs=cst[:OUT0, JH + 1:JH + 2],
                             start=False, stop=False)

            r1 = P.tile([128, JH * BS], f32, tag="r1")
            JD = JH // 2                 # first half on DVE, second on ACT
            for j in range(JH):
                sl = slice(j * BS, (j + 1) * BS)
                if j < JD:
                    nc.vector.tensor_scalar(r1[:, sl], h2[:], float(j), 0.0,
                                            A.subtract, A.max)
                else:
                    nc.scalar.activation(r1[:, sl], h2[:], AF.Relu,
                                         bias=float(-j))
            q1 = P.tile([128, JH * BS], f32, tag="q1")
            z1 = P.tile([128, JH * BS], f32, tag="z1")
            CH1 = 2 * BS  # chunk: 2 j-blocks
            nchunks = (JH + 1) // 2
            for ci in range(nchunks):
                sl = slice(ci * CH1, min((ci + 1) * CH1, JH * BS))
                if ci == nchunks - 1:
                    # last chunk on Pool (r1 is SBUF), overlapping ACT chunks
                    nc.gpsimd.tensor_tensor(out=q1[:, sl], in0=r1[:, sl],
                                            in1=r1[:, sl], op=A.mult)
                else:
                    nc.scalar.activation(q1[:, sl], r1[:, sl], AF.Square)
                nc.vector.tensor_tensor(out=z1[:, sl], in0=q1[:, sl],
                                        in1=r1[:, sl], op=A.mult)
            for j in range(JH):
                nc.tensor.matmul(out=yps[:],
                                 lhsT=z1[:, j * BS:(j + 1) * BS],
                                 rhs=cst[:, j:j + 1],
                                 start=False, stop=(j == JH - 1))

            osb = P.tile([BS, 1], f32, tag="osb")
            nc.scalar.activation(osb[:], yps[:], AF.Sigmoid, bias=float(bias1))
            nc.sync.dma_start(out=d_out[:], in_=osb[:])

    nc.compile()
    return nc


def kernel(
    user_indices, item_indices, grid_update_num, stop_grid_update_step,
    emb_user, emb_item,
    grid0, coef0, sb0, ssp0, bias0,
    grid1, coef1, sb1, ssp1, bias1,
):
    global LAST_RESULTS
    from concourse.bass_utils import run_bass_kernel_spmd

    uidx = np.asarray(user_indices).astype(np.int32).reshape(B_FULL, 1)
    iidx = np.asarray(item_indices).astype(np.int32).reshape(B_FULL, 1)
    eu = np.asarray(emb_user, dtype=np.float32)
    ei = np.asarray(emb_item, dtype=np.float32)
    embc = np.ascontiguousarray(
        np.concatenate([eu, ei], axis=0).astype(np.float16))
    x_min = float(min(eu.min(), ei.min()))
    x_max = float(max(eu.max(), ei.max()))

    key, w = _fold_host_weights(
        np.asarray(grid0, dtype=np.float32), np.asarray(coef0, dtype=np.float32),
        np.asarray(sb0, dtype=np.float32), np.asarray(ssp0, dtype=np.float32),
        np.asarray(bias0, dtype=np.float32), np.asarray(grid1, dtype=np.float32),
        np.asarray(coef1, dtype=np.float32), np.asarray(sb1, dtype=np.float32),
        np.asarray(ssp1, dtype=np.float32), np.asarray(bias1, dtype=np.float32),
        x_min, x_max,
    )

    cache_key = (key, w["blob"].shape[1])
    if cache_key not in _BUILD_CACHE:
        _BUILD_CACHE[cache_key] = _build_program(key, w["blob"].shape[1])
    nc = _BUILD_CACHE[cache_key]

    idxc = np.concatenate([uidx, iidx + NU], axis=1)  # (B, 2) int32
    in_maps = []
    for c in range(NCORES):
        sl = slice(c * BS, (c + 1) * BS)
        in_maps.append(
            {
                "idx": np.ascontiguousarray(idxc[sl]),
                "embc": embc,
                "blob": w["blob"],
                "biasrow": w["biasrow"],
                "consts": w["consts"],
            }
        )

    res = run_bass_kernel_spmd(nc, in_maps, core_ids=list(range(NCORES)),
                               trace=TRACE)
    LAST_RESULTS = res
    return np.concatenate([r["out"] for r in res.results], axis=0)


# revision 23
# speedup vs baseline: 1.1773x; 1.1773x over previous
"""Trainium2 Bass kernel for nn_KANModel (KAN recommender).

Math: with the shared uniform grid (G=5, k=3), cubic B-spline bases on the
extended uniform knots are shifted cardinal splines, so each KAN layer is a
sum of relu(u-n)^3 blocks folded into per-block matmul weights (see v1).
Two structural collapses on top of that:

  * layer 0: the gathered embeddings x span [x_min, x_max], so
    u0 = (x-t0)/h stays in ~[4.1, 6.8]. Blocks with n <= floor(u0_min)
    never clamp -> their relu cubes sum to a single cubic polynomial in
    v = u0 - vc, evaluated as three matmuls (v, v^2, v^3) with host-folded
    coefficients; only the top blocks keep their relu. silu(x) rides the
    same accumulation via sigmoid(x)*(v*h + t0v) split into two matmuls.

  * layer 0 outputs are produced feature-major and DUPLICATED across both
    partition halves (lhsT = [W | W]), pre-scaled by 1/h1. Layer 1's 12
    relu^3 blocks then pack two-per-partition-group: 6 elementwise ops with
    per-partition constant vectors + 6 f32 matmuls contracting (k,o) pairs
    over partitions into y[b].

Layer 0 runs in fp16 (verified ~7e-4 final rel err); layer 1 keeps f32
(r^3 reaches ~3e3 with heavy cancellation - fp16 fails the 2e-2 gate).
Embedding rows are fetched with ONE 256-descriptor indirect DMA from a
host-concatenated [NU+NI, 64] table (one SWDGE generation pass instead of
two serialized ones). Data-parallel over batch: 1024 rows -> 8 cores x 128.
"""

import numpy as np

B_FULL = 1024
NCORES = 8
BS = B_FULL // NCORES          # batch shard per core
D = 64                         # embedding dim
IN0, OUT0 = 2 * D, 64          # KAN layer 0
IN1 = 64                       # KAN layer 1 (out_dim 1)
G, KORD = 5, 3
NC_BASIS = G + KORD            # 8 spline bases per edge
NZ = G + 2 * KORD + 1          # 12 possible relu-cube shifts
NU, NI = 100000, 50000

_BUILD_CACHE = {}
TRACE = False
LAST_RESULTS = None

_A5 = np.array([1.0, -4.0, 6.0, -4.0, 1.0], dtype=np.float64) / 6.0


def _m3(s):
    """Cardinal cubic B-spline, exact (clamped) evaluation, float64."""
    s = np.minimum(s, 4.0)
    out = np.zeros_like(s)
    for m in range(4):
        r = np.maximum(s - m, 0.0)
        out += _A5[m] * r * r * r
    return out


def _fold_host_weights(grid0, coef0, sb0, ssp0, bias0, grid1, coef1, sb1, ssp1,
                       bias1, x_min, x_max):
    """O(params) host-side prep: folded weights with the cubic collapse for
    never-clamping blocks, duplicated feature-major layouts, and the packed
    layer-1 constant vectors."""
    h0 = float(grid0[0, -1] - grid0[0, 0]) / G
    t0_0 = float(grid0[0, 0]) - KORD * h0
    h1 = float(grid1[0, -1] - grid1[0, 0]) / G
    t0_1 = float(grid1[0, 0]) - KORD * h1
    inv_h1 = 1.0 / h1

    u0_min = (x_min - t0_0) / h0
    u0_max = (x_max - t0_0) / h0
    ncut = int(np.floor(u0_min + 1e-9))          # blocks 0..ncut never clamp
    vc = 0.5 * (u0_min + u0_max)                  # centering for fp16 powers
    t0v = t0_0 + vc * h0                          # v = (x - t0v)/h0
    nrem = [n for n in range(max(ncut + 1, 0), NZ) if n < u0_max + 1e-6]

    c0e = (ssp0[:, None].astype(np.float64) * coef0.astype(np.float64)).reshape(
        OUT0, IN0, NC_BASIS
    )  # (o, f, c)
    wz0 = np.zeros((NZ, IN0, OUT0), dtype=np.float64)
    for n in range(NZ):
        for m in range(5):
            c = n - m
            if 0 <= c < NC_BASIS:
                wz0[n] += _A5[m] * c0e[:, :, c].T

    # cubic collapse: sum_{n<=ncut} w_n (v + (vc-n))^3 -> powers of v
    cpoly = np.zeros((4, IN0, OUT0), dtype=np.float64)
    for n in range(ncut + 1):
        a = vc - n
        cpoly[0] += wz0[n] * a ** 3
        cpoly[1] += wz0[n] * (3 * a * a)
        cpoly[2] += wz0[n] * (3 * a)
        cpoly[3] += wz0[n]

    sb0e = sb0.reshape(OUT0, IN0).astype(np.float64)  # (o, f)

    def dup(w_fo):  # [f, OUT0] -> [f, 128] duplicated across halves
        return np.concatenate([w_fo, w_fo], axis=1)

    blocks = [dup(cpoly[1] * inv_h1), dup(cpoly[2] * inv_h1),
              dup(cpoly[3] * inv_h1)]
    for n in nrem:
        blocks.append(dup(wz0[n] * inv_h1))
    blocks.append(dup(sb0e.T * (h0 * inv_h1)))    # rhs = sg*v
    blocks.append(dup(sb0e.T * (t0v * inv_h1)))   # rhs = sg
    blob = np.ascontiguousarray(
        np.concatenate(blocks, axis=1).astype(np.float16))

    c0sum = cpoly[0].sum(axis=0)                  # constant term per o
    biasrow = np.ascontiguousarray(
        (np.tile(bias0.astype(np.float64) + c0sum, 2) * inv_h1)
        .reshape(1, 2 * OUT0).astype(np.float32))

    # ---- rigorous layer-0 output interval (grid + Lipschitz pad) ----
    NGRID = 2049
    xg = np.linspace(x_min, x_max, NGRID)
    dx = (x_max - x_min) / (NGRID - 1) if x_max > x_min else 0.0
    ug = (xg - t0_0) / h0
    basis = np.stack([_m3(ug - c) for c in range(NC_BASIS)], axis=1)
    silug = xg / (1.0 + np.exp(-xg))
    phi = sb0e[:, :, None] * silug[None, None, :] + np.einsum(
        "ofc,gc->ofg", c0e, basis
    )
    lip = np.abs(sb0e) * 1.1 + np.abs(c0e).sum(axis=2) * (0.75 / h0)
    pad = lip * dx
    h_max = bias0.astype(np.float64) + (phi.max(axis=2) + pad).sum(axis=1)
    u1_max = (float(h_max.max()) - t0_1) / h1
    nlist1 = [n for n in range(NZ) if n < u1_max + 1e-3]
    L1 = len(nlist1)
    JH = (L1 + 1) // 2                            # packed col-blocks

    c1e = ssp1[:, None].astype(np.float64) * coef1.astype(np.float64)
    wz1 = np.zeros((NZ, IN1), dtype=np.float64)
    for n in range(NZ):
        for m in range(5):
            c = n - m
            if 0 <= c < NC_BASIS:
                wz1[n] += _A5[m] * c1e[:, c]

    # Layer-1 packing pairs blocks (j, j+JH): the per-partition constant is
    # then shift(p) = t0_1/h1 + JH*(p>=64), j-independent, and folds into
    # the layer-0 bias row. Every r-block uses a PLAIN float scalar (per-
    # partition scalar APs serialize across engines in the tile scheduler).
    w1pack = np.zeros((2 * OUT0, JH), dtype=np.float64)
    for j in range(JH):
        for half in range(2):
            k = j + JH * half
            sl = slice(half * OUT0, (half + 1) * OUT0)
            if k < L1:
                w1pack[sl, j] = wz1[nlist1[k]]
    sb1h1 = np.zeros((2 * OUT0, 2), dtype=np.float64)
    sb1h1[:OUT0, 0] = sb1.astype(np.float64) * h1   # vs h2s*sg1
    sb1h1[:OUT0, 1] = sb1.astype(np.float64) * t0_1  # vs sg1 alone
    # consts layout: [w1pack JH | sb1h1 | sb1t01]
    consts = np.ascontiguousarray(
        np.concatenate([w1pack, sb1h1], axis=1).astype(np.float32))

    # fold the layer-1 shift into the bias row: h2s = u1 - JH*(p>=64)
    shift = t0_1 * inv_h1 + JH * (np.arange(2 * OUT0) >= OUT0)
    biasrow = np.ascontiguousarray(
        (biasrow.astype(np.float64) - shift).reshape(1, 2 * OUT0)
        .astype(np.float32))

    key = (
        t0v, 1.0 / h0, vc, float(h1), inv_h1, float(bias1[0]), float(t0_1),
        tuple(nrem), JH,
    )
    return key, dict(blob=blob, biasrow=biasrow, consts=consts)


def _build_program(key, ncols_blob):
    import concourse.bass as bass
    import concourse.bacc as bacc
    import concourse.mybir as mybir
    from concourse.tile import TileContext
    from concourse.masks import make_identity

    t0v, inv_h0, vc, h1, inv_h1, bias1, t0_1, nrem, JH = key
    NR = len(nrem)
    f32 = mybir.dt.float32
    f16 = mybir.dt.float16
    i32 = mybir.dt.int32
    A = mybir.AluOpType
    AF = mybir.ActivationFunctionType

    nc = bacc.Bacc("TRN2")

    # register float activation biases as const APs (mirrors Bass.__init__).
    # Only ACT uses these: const-AP biases on ACT don't hit the cross-engine
    # scalar-pointer serialization that explicit AP scalars do.
    cvals = {float(t0v), float(t0_1), float(bias1)}
    cvals.update(float(-(n - vc)) for n in nrem)      # layer-0 q' biases
    cvals.update(float(-j) for j in range(JH // 2, JH))  # layer-1 ACT relus
    for ci, val in enumerate(sorted(cvals)):
        if (f32, val) not in nc.const_aps.aps:
            t = nc.alloc_sbuf_tensor(f"kconst-{ci}", [128, 1], f32)
            nc.gpsimd.memset(t.ap(), val)
            nc.const_aps.aps[(f32, val)] = t.ap()

    d_idx = nc.dram_tensor("idx", [BS, 2], i32, kind="ExternalInput")
    d_emb = nc.dram_tensor("embc", [NU + NI, D], f16, kind="ExternalInput")
    d_blob = nc.dram_tensor("blob", [IN0, ncols_blob], f16, kind="ExternalInput")
    d_bias = nc.dram_tensor("biasrow", [1, 2 * OUT0], f32, kind="ExternalInput")
    d_cst = nc.dram_tensor("consts", [2 * OUT0, JH + 2], f32,
                           kind="ExternalInput")
    d_out = nc.dram_tensor("out", [BS, 1], f32, kind="ExternalOutput")

    with TileContext(nc) as tc:
        with (
            tc.tile_pool(name="sb", bufs=1) as P,
            tc.tile_pool(name="ps", bufs=1, space="PSUM") as PS,
        ):
            # idx first on SP: its completion gates the gather desc-gens
            idx = P.tile([BS, 2], i32, tag="idx")
            nc.sync.dma_start(out=idx[:], in_=d_idx[:])
            blob = P.tile([IN0, ncols_blob], f16, tag="blob")
            nc.sync.dma_start(out=blob[:], in_=d_blob[:])
            brow = P.tile([1, 2 * OUT0], f32, tag="brow")
            nc.sync.dma_start(out=brow[:1, :], in_=d_bias[:])
            cst = P.tile([2 * OUT0, JH + 2], f32, tag="cst")
            nc.sync.dma_start(out=cst[:], in_=d_cst[:])

            ident = P.tile([128, 128], f16, tag="ident")
            make_identity(nc, ident[:])
            ones = P.tile([1, 128], f32, tag="ones")
            nc.gpsimd.memset(ones[:1, :], 1.0)
            # dummy ACT op: charges the 1283ns activation-table load at t~1us
            # instead of on the critical path at first real use (Sigmoid set
            # also contains Square and Relu -> single load for the kernel)
            warm = P.tile([1, 1], f32, tag="warm")
            nc.scalar.activation(warm[:1, :1], ones[:1, 0:1], AF.Sigmoid)

            # two row gathers from the fp16 combined table (multi-chunk
            # offset APs are not supported by the SWDGE ucode)
            xbm = P.tile([BS, 2, D], f16, tag="xbm")
            nc.gpsimd.indirect_dma_start(
                out=xbm[:, 0, :], out_offset=None, in_=d_emb[:],
                in_offset=bass.IndirectOffsetOnAxis(ap=idx[:, 0:1], axis=0),
            )
            nc.gpsimd.indirect_dma_start(
                out=xbm[:, 1, :], out_offset=None, in_=d_emb[:],
                in_offset=bass.IndirectOffsetOnAxis(ap=idx[:, 1:2], axis=0),
            )

            # h2 accumulation [o-dup, b]; bias matmul first (operands land
            # early; PE p-state warms before the gather-dependent chain)
            h2 = PS.tile([128, BS], f32, tag="h2")
            nc.tensor.matmul(out=h2[:], lhsT=brow[:1, :], rhs=ones[:1, :],
                             start=True, stop=False)

            # per-half transposes to feature-major x^T (f16, 1 cyc/row);
            # the user half transposes while the item gather is in flight
            xT = PS.tile([128, BS], f16, tag="xT")
            nc.tensor.matmul(out=xT[0:D, :], lhsT=xbm[:, 0, :], rhs=ident[:],
                             is_transpose=True, start=True, stop=True,
                             skip_group_check=True)
            nc.tensor.matmul(out=xT[D:128, :], lhsT=xbm[:, 1, :], rhs=ident[:],
                             is_transpose=True, start=True, stop=True,
                             skip_group_check=True)

            # fp16 operands for the layer-0 matmuls; v is the ONLY xT reader.
            # DVE: v, v2, v3, r-blocks, sv, z-blocks; ACT: sg + q'-blocks
            # (q' = (v - c)^2 straight from v via const-AP bias: no r->q dep)
            v = P.tile([128, BS], f16, tag="v")
            nc.vector.tensor_scalar(v[:], xT[:], t0v, inv_h0,
                                    A.subtract, A.mult)
            sg = P.tile([128, BS], f16, tag="sg")
            nc.scalar.activation(sg[:], v[:], AF.Sigmoid, bias=t0v,
                                 scale=1.0 / inv_h0)
            v2 = P.tile([128, BS], f16, tag="v2")
            nc.vector.tensor_tensor(out=v2[:], in0=v[:], in1=v[:], op=A.mult)
            v3 = P.tile([128, BS], f16, tag="v3")
            nc.vector.tensor_tensor(out=v3[:], in0=v2[:], in1=v[:], op=A.mult)

            r56 = P.tile([128, NR * BS], f16, tag="r56")
            q56 = P.tile([128, NR * BS], f16, tag="q56")
            z56 = P.tile([128, NR * BS], f16, tag="z56")
            for i, n in enumerate(nrem):
                sl = slice(i * BS, (i + 1) * BS)
                nc.vector.tensor_scalar(r56[:, sl], v[:],
                                        float(n - vc), 0.0, A.subtract, A.max)
                nc.scalar.activation(q56[:, sl], v[:], AF.Square,
                                     bias=float(-(n - vc)))
            sv = P.tile([128, BS], f16, tag="sv")
            nc.vector.tensor_tensor(out=sv[:], in0=sg[:], in1=v[:], op=A.mult)
            for i in range(NR):
                sl = slice(i * BS, (i + 1) * BS)
                nc.vector.tensor_tensor(out=z56[:, sl], in0=q56[:, sl],
                                        in1=r56[:, sl], op=A.mult)

            # z-blocks come last in PE order: their operands are ready last
            mm_list = []
            for bi, t in enumerate([v, v2, v3]):
                mm_list.append((blob[:, bi * 128:(bi + 1) * 128], t[:]))
            mm_list.append((blob[:, (3 + NR) * 128:(4 + NR) * 128], sv[:]))
            mm_list.append((blob[:, (4 + NR) * 128:(5 + NR) * 128], sg[:]))
            for i in range(NR):
                mm_list.append((blob[:, (3 + i) * 128:(4 + i) * 128],
                                z56[:, i * BS:(i + 1) * BS]))
            for i, (w_ap, rhs_ap) in enumerate(mm_list):
                nc.tensor.matmul(out=h2[:], lhsT=w_ap, rhs=rhs_ap,
                                 start=False, stop=(i == len(mm_list) - 1))

            # ---- layer 1, packed feature-major ----
            # h2 holds u1 - JH*(p>=64). One PSUM reader: copy to SBUF (PSUM-
            # sourced ops pay ~160ns dispatch gaps; SBUF ops run back-to-back)
            # then fan the r/q/z pipeline across DVE, ACT, and Pool.
            h2s = P.tile([128, BS], f32, tag="h2s")
            nc.vector.tensor_scalar(h2s[:], h2[:], 0.0, None, A.add)

            r1 = P.tile([128, JH * BS], f32, tag="r1")
            q1 = P.tile([128, JH * BS], f32, tag="q1")
            z1 = P.tile([128, JH * BS], f32, tag="z1")

            def rblk(j, eng):
                sl = slice(j * BS, (j + 1) * BS)
                if eng is nc.scalar:
                    nc.scalar.activation(r1[:, sl], h2s[:], AF.Relu,
                                         bias=float(-j))
                else:
                    eng.tensor_scalar(r1[:, sl], h2s[:], float(j), 0.0,
                                      A.subtract, A.max)

            # DVE: copy, r0-r2, sv1, z1a, z1b | ACT: sg1, r3, q1a, q1b
            # Pool: r4, r5, q1c, z1c
            sg1 = P.tile([OUT0, BS], f32, tag="sg1")
            nc.scalar.activation(sg1[:], h2s[:OUT0, :], AF.Sigmoid,
                                 bias=float(t0_1), scale=h1)
            for j in (0, 1, 2):
                rblk(j, nc.vector)
            rblk(3, nc.scalar)
            rblk(4, nc.gpsimd)
            rblk(5, nc.gpsimd)

            # silu path: h = h2s[:64]*h1 + t0_1, so
            # silu(h)*sb1 = (h2s*sg1)*(sb1*h1) + sg1*(sb1*t0_1)
            sv1 = P.tile([OUT0, BS], f32, tag="sv1")
            nc.vector.tensor_tensor(out=sv1[:], in0=sg1[:], in1=h2s[:OUT0, :],
                                    op=A.mult)
            yps = PS.tile([BS, 1], f32, tag="yps")
            nc.tensor.matmul(out=yps[:], lhsT=sv1[:],
                             rhs=cst[:OUT0, JH:JH + 1],
                             start=True, stop=False)
            nc.tensor.matmul(out=yps[:], lhsT=sg1[:],
                             rhs=cst[:OUT0, JH + 1:JH + 2],
                             start=False, stop=False)

            CH1 = 2 * BS  # chunk: 2 j-blocks
            qeng = [nc.scalar, nc.scalar, nc.gpsimd]
            zeng = [nc.vector, nc.vector, nc.gpsimd]
            for ci in range((JH + 1) // 2):
                sl = slice(ci * CH1, min((ci + 1) * CH1, JH * BS))
                if qeng[ci] is nc.scalar:
                    nc.scalar.activation(q1[:, sl], r1[:, sl], AF.Square)
                else:
                    qeng[ci].tensor_tensor(out=q1[:, sl], in0=r1[:, sl],
                                           in1=r1[:, sl], op=A.mult)
                zeng[ci].tensor_tensor(out=z1[:, sl], in0=q1[:, sl],
                                       in1=r1[:, sl], op=A.mult)
            for j in range(JH):
                nc.tensor.matmul(out=yps[:],
                                 lhsT=z1[:, j * BS:(j + 1) * BS],
                                 rhs=cst[:, j:j + 1],
                                 start=False, stop=(j == JH - 1))

            osb = P.tile([BS, 1], f32, tag="osb")
            nc.scalar.activation(osb[:], yps[:], AF.Sigmoid, bias=float(bias1))
            nc.sync.dma_start(out=d_out[:], in_=osb[:])

    nc.compile()
    return nc


def kernel(
    user_indices, item_indices, grid_update_num, stop_grid_update_step,
    emb_user, emb_item,
    grid0, coef0, sb0, ssp0, bias0,
    grid1, coef1, sb1, ssp1, bias1,
):
    global LAST_RESULTS
    from concourse.bass_utils import run_bass_kernel_spmd

    uidx = np.asarray(user_indices).astype(np.int32).reshape(B_FULL, 1)
    iidx = np.asarray(item_indices).astype(np.int32).reshape(B_FULL, 1)
    eu = np.asarray(emb_user, dtype=np.float32)
    ei = np.asarray(emb_item, dtype=np.float32)
    embc = np.ascontiguousarray(
        np.concatenate([eu, ei], axis=0).astype(np.float16))
    x_min = float(min(eu.min(), ei.min()))
    x_max = float(max(eu.max(), ei.max()))

    key, w = _fold_host_weights(
        np.asarray(grid0, dtype=np.float32), np.asarray(coef0, dtype=np.float32),
        np.asarray(sb0, dtype=np.float32), np.asarray(ssp0, dtype=np.float32),
        np.asarray(bias0, dtype=np.float32), np.asarray(grid1, dtype=np.float32),
        np.asarray(coef1, dtype=np.float32), np.asarray(sb1, dtype=np.float32),
        np.asarray(ssp1, dtype=np.float32), np.asarray(bias1, dtype=np.float32),
        x_min, x_max,
    )

    cache_key = (key, w["blob"].shape[1])
    if cache_key not in _BUILD_CACHE:
        _BUILD_CACHE[cache_key] = _build_program(key, w["blob"].shape[1])
    nc = _BUILD_CACHE[cache_key]

    idxc = np.concatenate([uidx, iidx + NU], axis=1)  # (B, 2) int32
    in_maps = []
    for c in range(NCORES):
        sl = slice(c * BS, (c + 1) * BS)
        in_maps.append(
            {
                "idx": np.ascontiguousarray(idxc[sl]),
                "embc": embc,
                "blob": w["blob"],
                "biasrow": w["biasrow"],
                "consts": w["consts"],
            }
        )

    res = run_bass_kernel_spmd(nc, in_maps, core_ids=list(range(NCORES)),
                               trace=TRACE)
    LAST_RESULTS = res
    return np.concatenate([r["out"] for r in res.results], axis=0)
